# revision 2
# baseline (speedup 1.0000x reference)
"""DeepSet GNN message-passing kernel for 8 TRN2 NeuronCores.

Strategy:
  - segment_ids are sorted, so shard by *segment windows*: 392 windows of 128
    segments, 49 windows per core. Each core handles exactly the neighbor rows
    whose segment falls in its windows -> no cross-core reduction at all.
  - Host folds phi_w2 past the segment sum (segment_sum(h@W2+b2) =
    segment_sum(h)@W2 + counts*b2), transposes neighbors to fp16 [64, N] and
    pads each window's rows to a fixed B blocks of 128 rows so all 8 cores run
    one identical (SPMD) program.
  - Device per 128-row block: self-loading matmul h1 = relu-able (X_aug @ W1_aug)
    into PSUM, ACT relu-copy to fp16, DVE builds a one-hot [row, seg] via
    iota/is_equal against the row's local segment id, PE accumulates
    one_hot.T @ h1 into a per-window PSUM tile [128 segs, 64].
  - Per window: tiny rho MLP chain on PE/ACT (transpose via PE identity,
    biases and the counts*b2 term folded in via augmented rows), output
    written transposed [2, segs]; host re-transposes.
"""

import sys

sys.path.insert(0, "/opt/trn_rl_repo")

import numpy as np

N_AGENTS = 50000
N_NEIGH = 1600000
D = 64
N_CORES = 8
SEG_T = 128  # segments per window (= PSUM partition dim)
W_PER_CORE = 49
NW_TOT = N_CORES * W_PER_CORE  # 392 windows of 128 segs >= 50000
SEGS_PER_CORE = W_PER_CORE * SEG_T  # 6272


def _build_program(B):
    """Build the SPMD bacc program for B blocks (of 128 rows) per window."""
    from concourse import bacc, mybir
    import concourse.tile as tile

    FP16 = mybir.dt.float16
    F32 = mybir.dt.float32
    Relu = mybir.ActivationFunctionType.Relu
    Copy = mybir.ActivationFunctionType.Copy

    NBLK = W_PER_CORE * B
    NCOL = NBLK * 128

    nc = bacc.Bacc("TRN2", target_bir_lowering=False, debug=False)
    xta = nc.dram_tensor("xta", [65, NCOL], FP16, kind="ExternalInput").ap()
    qid = nc.dram_tensor("qid", [128, NBLK], F32, kind="ExternalInput").ap()
    cnt = nc.dram_tensor("cnt", [2, SEGS_PER_CORE], FP16, kind="ExternalInput").ap()
    w1a = nc.dram_tensor("w1a", [65, 64], FP16, kind="ExternalInput").ap()
    waa = nc.dram_tensor("waa", [66, 64], FP16, kind="ExternalInput").ap()
    wba = nc.dram_tensor("wba", [65, 2], FP16, kind="ExternalInput").ap()
    iota = nc.dram_tensor("iota", [128, 128], FP16, kind="ExternalInput").ap()
    iden = nc.dram_tensor("iden", [128, 128], FP16, kind="ExternalInput").ap()
    out = nc.dram_tensor("out", [2, SEGS_PER_CORE], F32, kind="ExternalOutput").ap()

    with tile.TileContext(nc) as tc:
        with (
            tc.tile_pool(name="const", bufs=1) as cpool,
            tc.tile_pool(name="x", bufs=6) as xpool,
            tc.tile_pool(name="h", bufs=4) as hpool,
            tc.tile_pool(name="oh", bufs=4) as ohpool,
            tc.tile_pool(name="rho", bufs=2) as rpool,
            tc.tile_pool(name="psh", bufs=3, space="PSUM") as psh,
            tc.tile_pool(name="pss", bufs=2, space="PSUM") as pss,
            tc.tile_pool(name="pst", bufs=1, space="PSUM") as pst,
            tc.tile_pool(name="psr", bufs=1, space="PSUM") as psr,
            tc.tile_pool(name="pso", bufs=1, space="PSUM") as pso,
        ):
            w1a_t = cpool.tile([65, 64], FP16)
            nc.sync.dma_start(w1a_t[:], w1a[:, :])
            waa_t = cpool.tile([66, 64], FP16)
            nc.sync.dma_start(waa_t[:], waa[:, :])
            wba_t = cpool.tile([65, 2], FP16)
            nc.sync.dma_start(wba_t[:], wba[:, :])
            iota_t = cpool.tile([128, 128], FP16)
            nc.sync.dma_start(iota_t[:], iota[:, :])
            iden_t = cpool.tile([128, 128], FP16)
            nc.sync.dma_start(iden_t[:], iden[:, :])
            # all per-block segment ids, loaded once: [128, NBLK] f32
            qall_t = cpool.tile([128, NBLK], F32)
            nc.sync.dma_start(qall_t[:], qid[:, :])

            for w in range(W_PER_CORE):
                s_ps = pss.tile([128, 64], F32)
                for j in range(0, B, 2):
                    col0 = 128 * (B * w + j)
                    xt = xpool.tile([65, 256], FP16)
                    nc.sync.dma_start(xt[:], xta[:, col0 : col0 + 256])
                    for k in range(2):
                        b = j + k
                        hp = psh.tile([128, 64], F32)
                        nc.tensor.matmul(
                            hp[:], lhsT=xt[:, 128 * k : 128 * k + 128],
                            rhs=w1a_t[:], start=True, stop=True,
                        )
                        hs = hpool.tile([128, 64], FP16)
                        nc.scalar.activation(hs[:], hp[:], Relu)
                        oh = ohpool.tile([128, 128], FP16)
                        nc.vector.tensor_scalar(
                            out=oh[:], in0=iota_t[:],
                            scalar1=qall_t[:, B * w + b : B * w + b + 1],
                            scalar2=0.0,
                            op0=mybir.AluOpType.subtract,
                            op1=mybir.AluOpType.is_equal,
                        )
                        nc.tensor.matmul(
                            s_ps[:], lhsT=oh[:], rhs=hs[:],
                            start=(b == 0), stop=(b == B - 1),
                        )
                # rho MLP on the window's 128 pooled segments
                s_sb = rpool.tile([128, 64], FP16)
                nc.scalar.activation(s_sb[:], s_ps[:], Copy)
                st_ps = pst.tile([64, 128], FP16)
                nc.tensor.transpose(st_ps[:], s_sb[:], iden_t[:])
                st_sb = rpool.tile([66, 128], FP16)
                nc.vector.tensor_copy(st_sb[0:64, :], st_ps[:])
                nc.sync.dma_start(
                    st_sb[64:66, :], cnt[:, SEG_T * w : SEG_T * w + SEG_T]
                )
                r_ps = psr.tile([64, 128], F32)
                nc.tensor.matmul(r_ps[:], lhsT=waa_t[:], rhs=st_sb[:], start=True, stop=True)
                r_sb = rpool.tile([65, 128], FP16)
                nc.scalar.activation(r_sb[0:64, :], r_ps[:], Relu)
                nc.sync.dma_start(
                    r_sb[64:65, :], cnt[1:2, SEG_T * w : SEG_T * w + SEG_T]
                )
                o_ps = pso.tile([2, 128], F32)
                nc.tensor.matmul(o_ps[:], lhsT=wba_t[:], rhs=r_sb[:], start=True, stop=True)
                o_sb = rpool.tile([2, 128], F32)
                nc.vector.tensor_copy(o_sb[:], o_ps[:])
                nc.sync.dma_start(out[:, SEG_T * w : SEG_T * w + SEG_T], o_sb[:])
    nc.compile()
    return nc


def _host_prep(neighbors, phi_w1, phi_b1, phi_w2, phi_b2,
               rho_w1, rho_b1, rho_w2, rho_b2, segment_ids):
    ids = np.asarray(segment_ids)
    X = np.asarray(neighbors)

    bounds = np.minimum(np.arange(NW_TOT + 1) * SEG_T, N_AGENTS)
    edges = np.searchsorted(ids, bounds)  # row range per window
    rows_w = np.diff(edges)
    B = int(np.ceil(rows_w.max() / 128))
    B += B % 2  # even so we can DMA two blocks at a time

    NBLK = W_PER_CORE * B
    NCOL = NBLK * 128

    XT = np.ascontiguousarray(X.T).astype(np.float16)  # [64, N]
    counts = np.bincount(ids, minlength=NW_TOT * SEG_T).astype(np.float16)

    in_maps = []
    consts = dict(
        w1a=np.concatenate([phi_w1, phi_b1[None, :]], 0).astype(np.float16),
        waa=np.concatenate(
            [phi_w2 @ rho_w1, (phi_b2 @ rho_w1)[None, :], rho_b1[None, :]], 0
        ).astype(np.float16),
        wba=np.concatenate([rho_w2, rho_b2[None, :]], 0).astype(np.float16),
        iota=np.tile(np.arange(128, dtype=np.float16), (128, 1)),
        iden=np.eye(128, dtype=np.float16),
    )
    for c in range(N_CORES):
        xta = np.zeros((65, NCOL), np.float16)
        qflat = np.full(NCOL, -1.0, np.float32)
        for wl in range(W_PER_CORE):
            wg = W_PER_CORE * c + wl
            a, e = edges[wg], edges[wg + 1]
            n = e - a
            c0 = wl * B * 128
            xta[0:64, c0 : c0 + n] = XT[:, a:e]
            xta[64, c0 : c0 + n] = 1.0
            qflat[c0 : c0 + n] = (ids[a:e] - SEG_T * wg).astype(np.float32)
        qid = np.ascontiguousarray(qflat.reshape(NBLK, 128).T)  # [128, NBLK]
        cnt = np.empty((2, SEGS_PER_CORE), np.float16)
        cnt[0] = counts[SEGS_PER_CORE * c : SEGS_PER_CORE * (c + 1)]
        cnt[1] = 1.0
        in_maps.append(dict(xta=xta, qid=qid, cnt=cnt, **consts))
    return B, in_maps


LAST_RESULTS = None


def kernel(**inputs):
    global LAST_RESULTS
    np_inputs = {k: np.asarray(v) for k, v in inputs.items()}
    B, in_maps = _host_prep(**np_inputs)
    nc = _build_program(B)

    from concourse.bass_utils import run_bass_kernel_spmd

    res = run_bass_kernel_spmd(nc, in_maps, list(range(N_CORES)))
    LAST_RESULTS = res
    out_t = np.concatenate(
        [res.results[c]["out"] for c in range(N_CORES)], axis=1
    )  # [2, 50176]
    return np.ascontiguousarray(out_t[:, :N_AGENTS].T).astype(np.float32)



# revision 12
# speedup vs baseline: 4.9356x; 4.9356x over previous
"""DeepSet GNN message-passing kernel for 8 TRN2 NeuronCores.

Strategy (v2):
  - segment_ids are sorted; shard by contiguous segment ranges: core c owns
    segments [c*6250, (c+1)*6250) and exactly the neighbor rows mapping there
    -> no cross-core reduction.
  - All-transposed dataflow: device computes h^T = relu(W1^T X^T + b1) with
    W1 STATIONARY on the PE (free dim = 512 neighbor columns per matmul).
    Two independent column streams are stacked on partition halves (rows
    0-63 / 64-127) and processed by ONE matmul via a block-diagonal
    [[W1,0],[0,W1]] stationary, so each 512-wide matmul handles 1024
    neighbors.
  - The per-segment sum is done as compile-time-scheduled windowed
    reductions on DVE (tensor_reduce over the innermost axis): the host
    sorts each core's segments by their exact neighbor count so equal-length
    runs are contiguous; the run schedule is shared across cores (max count
    per class over cores, pad slots zero-filled) so one SPMD program works
    for all 8 cores.
  - rho MLP: phi_w2 is folded past the segment sum (waa = phi_w2 @ rho_w1,
    count * (phi_b2 @ rho_w1) added via a rank-1 matmul with the counts
    row), biases via ACT per-partition bias / rank-1 matmuls. Output is
    written transposed; host un-permutes.
"""

import sys

sys.path.insert(0, "/opt/trn_rl_repo")

import numpy as np

N_AGENTS = 50000
N_NEIGH = 1600000
D = 64
N_CORES = 8
SPC = N_AGENTS // N_CORES  # 6250 segments per core
TILE_COLS = 2048  # columns per DMA/hs tile (per half-stream)
SUB = 512  # columns per matmul / PSUM bank
RHO_W = 512  # segments per rho window

LAST_RESULTS = None


def _make_schedule(counts):
    """counts: [N_CORES, SPC] per-core segment sizes. Build the shared
    column schedule: for each distinct segment size k (ascending), allocate
    ceil(max_core_count(k)/2) slots of length max(k,1); slots never cross a
    TILE_COLS boundary. Returns run lists per tile plus slot metadata."""
    KMAX = int(counts.max())
    n_prog = np.zeros(KMAX + 1, np.int64)
    for c in range(N_CORES):
        bc = np.bincount(counts[c], minlength=KMAX + 1)
        n_prog = np.maximum(n_prog, bc)
    runs_by_tile = {}
    slot_col = []
    cls_slot0 = {}
    col = 0
    for kv in range(KMAX + 1):
        W = -(-int(n_prog[kv]) // 2)
        if W == 0:
            continue
        L = max(kv, 1)
        cls_slot0[kv] = len(slot_col)
        left = W
        while left:
            space = TILE_COLS - (col % TILE_COLS)
            nfit = min(left, space // L)
            if nfit == 0:
                col += space
                continue
            t = col // TILE_COLS
            runs_by_tile.setdefault(t, []).append(
                (col - t * TILE_COLS, nfit, L, len(slot_col))
            )
            slot_col.extend((col + L * np.arange(nfit)).tolist())
            col += nfit * L
            left -= nfit
    NSLOT = len(slot_col)
    NSLOT_pad = -(-NSLOT // RHO_W) * RHO_W
    NTILES = -(-col // TILE_COLS)
    S = NTILES * TILE_COLS
    return runs_by_tile, np.asarray(slot_col, np.int64), cls_slot0, NSLOT, NSLOT_pad, S, NTILES


def _build_program(runs_by_tile, NTILES, NSLOT_pad, S):
    from concourse import bacc, mybir
    import concourse.tile as tile

    FP16 = mybir.dt.float16
    F32 = mybir.dt.float32
    Relu = mybir.ActivationFunctionType.Relu
    Copy = mybir.ActivationFunctionType.Copy
    AXX = mybir.AxisListType.X
    ADD = mybir.AluOpType.add

    nc = bacc.Bacc("TRN2", target_bir_lowering=False, debug=False)
    xd = nc.dram_tensor("xd", [128, S], FP16, kind="ExternalInput").ap()
    cntd = nc.dram_tensor("cntd", [65, NSLOT_pad], FP16, kind="ExternalInput").ap()
    w2sd = nc.dram_tensor("w2sd", [128, 128], FP16, kind="ExternalInput").ap()
    b1d = nc.dram_tensor("b1d", [128, 1], F32, kind="ExternalInput").ap()
    waad = nc.dram_tensor("waad", [128, 64], FP16, kind="ExternalInput").ap()
    b2rd = nc.dram_tensor("b2rd", [65, 64], FP16, kind="ExternalInput").ap()
    rb1d = nc.dram_tensor("rb1d", [64, 1], F32, kind="ExternalInput").ap()
    rw2d = nc.dram_tensor("rw2d", [64, 2], FP16, kind="ExternalInput").ap()
    rb2d = nc.dram_tensor("rb2d", [1, 2], FP16, kind="ExternalInput").ap()
    onesd = nc.dram_tensor("onesd", [1, RHO_W], FP16, kind="ExternalInput").ap()
    outd = nc.dram_tensor("outd", [4, NSLOT_pad], F32, kind="ExternalOutput").ap()


    NWIN = NSLOT_pad // RHO_W
    with tile.TileContext(nc) as tc:
        with (
            tc.tile_pool(name="const", bufs=1) as cpool,
            tc.tile_pool(name="x", bufs=3) as xpool,
            tc.tile_pool(name="h", bufs=3) as hpool,
            tc.tile_pool(name="r", bufs=2) as rpool,
            tc.tile_pool(name="ph", bufs=4, space="PSUM") as ph,
            tc.tile_pool(name="pr", bufs=2, space="PSUM") as pr,
            tc.tile_pool(name="po", bufs=2, space="PSUM") as po,
        ):
            w2s_t = cpool.tile([128, 128], FP16)
            nc.sync.dma_start(w2s_t[:], w2sd[:, :])
            b1_t = cpool.tile([128, 1], F32)
            nc.sync.dma_start(b1_t[:], b1d[:, :])
            waa_t = cpool.tile([128, 64], FP16)
            nc.sync.dma_start(waa_t[:], waad[:, :])
            b2r_t = cpool.tile([65, 64], FP16)
            nc.sync.dma_start(b2r_t[:], b2rd[:, :])
            rb1_t = cpool.tile([64, 1], F32)
            nc.sync.dma_start(rb1_t[:], rb1d[:, :])
            rw2_t = cpool.tile([64, 2], FP16)
            nc.sync.dma_start(rw2_t[:], rw2d[:, :])
            rb2_t = cpool.tile([1, 2], FP16)
            nc.sync.dma_start(rb2_t[:], rb2d[:, :])
            ones_t = cpool.tile([1, RHO_W], FP16)
            nc.sync.dma_start(ones_t[:], onesd[:, :])
            cnt_t = cpool.tile([65, NSLOT_pad], FP16)
            nc.sync.dma_start(cnt_t[:], cntd[:, :])
            ssb = cpool.tile([128, NSLOT_pad], FP16)
            outsb = cpool.tile([66, NSLOT_pad], F32)
            nc.gpsimd.memset(ssb[:], 0.0)

            with nc.allow_low_precision(reason="fp16 segment sums within tol"):
                for t in range(NTILES):
                    xt = xpool.tile([128, TILE_COLS], FP16)
                    nc.sync.dma_start(
                        xt[:], xd[:, t * TILE_COLS : (t + 1) * TILE_COLS]
                    )
                    hst = hpool.tile([128, TILE_COLS], FP16)
                    for j in range(TILE_COLS // SUB):
                        hp = ph.tile([128, SUB], F32)
                        nc.tensor.matmul(
                            hp[:],
                            lhsT=w2s_t[:],
                            rhs=xt[:, j * SUB : (j + 1) * SUB],
                            start=True,
                            stop=True,
                        )
                        nc.scalar.activation(
                            hst[:, j * SUB : (j + 1) * SUB], hp[:], Relu,
                            bias=b1_t[:],
                        )
                    for (off, n, L, slot0) in runs_by_tile.get(t, []):
                        nc.vector.tensor_reduce(
                            ssb[:, slot0 : slot0 + n],
                            hst[:, off : off + n * L].rearrange(
                                "p (n l) -> p n l", l=L
                            ),
                            axis=AXX,
                            op=ADD,
                        )
                for w in range(NWIN):
                    c0 = w * RHO_W
                    for h in range(2):
                        rp = pr.tile([64, RHO_W], F32)
                        nc.tensor.matmul(
                            rp[:], lhsT=waa_t[64 * h : 64 * h + 64, :],
                            rhs=ssb[64 * h : 64 * h + 64, c0 : c0 + RHO_W],
                            start=True, stop=False,
                        )
                        nc.tensor.matmul(
                            rp[:], lhsT=b2r_t[64 * h : 64 * h + 1, :],
                            rhs=cnt_t[64 * h : 64 * h + 1, c0 : c0 + RHO_W],
                            start=False, stop=True,
                        )
                        rsb = rpool.tile([64, RHO_W], FP16)
                        nc.scalar.activation(rsb[:], rp[:], Relu, bias=rb1_t[:])
                        op_ = po.tile([2, RHO_W], F32)
                        nc.tensor.matmul(
                            op_[:], lhsT=rw2_t[:], rhs=rsb[:],
                            start=True, stop=False,
                        )
                        nc.tensor.matmul(
                            op_[:], lhsT=rb2_t[:], rhs=ones_t[:],
                            start=False, stop=True,
                        )
                        nc.scalar.activation(
                            outsb[64 * h : 64 * h + 2, c0 : c0 + RHO_W], op_[:],
                            Copy,
                        )
                    nc.sync.dma_start(
                        outd[0:2, c0 : c0 + RHO_W], outsb[0:2, c0 : c0 + RHO_W]
                    )
                    nc.sync.dma_start(
                        outd[2:4, c0 : c0 + RHO_W], outsb[64:66, c0 : c0 + RHO_W]
                    )
    nc.compile()
    return nc


def _host_prep(neighbors, phi_w1, phi_b1, phi_w2, phi_b2,
               rho_w1, rho_b1, rho_w2, rho_b2, segment_ids):
    ids = np.asarray(segment_ids)
    X16 = np.asarray(neighbors).astype(np.float16)

    edges = np.searchsorted(ids, np.arange(N_CORES + 1) * SPC)
    counts = np.zeros((N_CORES, SPC), np.int64)
    for c in range(N_CORES):
        counts[c] = np.bincount(ids[edges[c] : edges[c + 1]] - c * SPC,
                                minlength=SPC)
    sched = _make_schedule(counts)
    runs_by_tile, slot_col, cls_slot0, NSLOT, NSLOT_pad, S, NTILES = sched

    w2s = np.zeros((128, 128), np.float16)
    w2s[0:64, 0:64] = phi_w1
    w2s[64:128, 64:128] = phi_w1
    consts = dict(
        w2sd=w2s,
        b1d=np.tile(np.asarray(phi_b1, np.float32)[:, None], (2, 1)),
        waad=np.tile((np.asarray(phi_w2) @ np.asarray(rho_w1)).astype(np.float16),
                     (2, 1)),
        b2rd=np.zeros((65, 64), np.float16),
        rb1d=np.asarray(rho_b1, np.float32)[:, None],
        rw2d=np.asarray(rho_w2).astype(np.float16),
        rb2d=np.asarray(rho_b2)[None, :].astype(np.float16),
        onesd=np.ones((1, RHO_W), np.float16),
    )
    b2row = (np.asarray(phi_b2) @ np.asarray(rho_w1)).astype(np.float16)
    consts["b2rd"][0] = b2row
    consts["b2rd"][64] = b2row

    in_maps = []
    slotmaps = []
    for c in range(N_CORES):
        k = counts[c]
        rs = edges[c] + np.concatenate([[0], np.cumsum(k)])
        order = np.argsort(k, kind="stable")
        ksort = k[order]
        uniq, first = np.unique(ksort, return_index=True)
        xdv = np.zeros((128, S), np.float16)
        cnt = np.zeros((65, NSLOT_pad), np.float16)
        smap = np.full((2, NSLOT_pad), -1, np.int64)
        for h in (0, 1):
            seg_list, slot_list = [], []
            for i, kv in enumerate(uniq):
                s0 = first[i]
                s1 = first[i + 1] if i + 1 < len(uniq) else SPC
                segs = order[s0:s1]
                nE = (len(segs) + 1) // 2
                mine = segs[:nE] if h == 0 else segs[nE:]
                if len(mine) == 0:
                    continue
                base = cls_slot0[int(kv)]
                seg_list.append(mine)
                slot_list.append(base + np.arange(len(mine)))
            segs_f = np.concatenate(seg_list)
            slots_f = np.concatenate(slot_list)
            kk = k[segs_f]
            nz = kk > 0
            segs_nz, slots_nz, kknz = segs_f[nz], slots_f[nz], kk[nz]
            src0 = rs[segs_nz]
            col0 = slot_col[slots_nz]
            tot = int(kknz.sum())
            ofs = np.arange(tot) - np.repeat(np.cumsum(kknz) - kknz, kknz)
            src = np.repeat(src0, kknz) + ofs
            dst = np.repeat(col0, kknz) + ofs
            xdv[64 * h : 64 * h + 64, dst] = X16[src].T
            cnt[64 * h, slots_f] = kk.astype(np.float16)
            smap[h, slots_f] = segs_f + c * SPC
        in_maps.append(dict(xd=xdv, cntd=cnt, **consts))
        slotmaps.append(smap)
    return sched, in_maps, slotmaps


def kernel(**inputs):
    global LAST_RESULTS
    np_inputs = {kk: np.asarray(v) for kk, v in inputs.items()}
    sched, in_maps, slotmaps = _host_prep(**np_inputs)
    runs_by_tile, slot_col, cls_slot0, NSLOT, NSLOT_pad, S, NTILES = sched
    nc = _build_program(runs_by_tile, NTILES, NSLOT_pad, S)

    from concourse.bass_utils import run_bass_kernel_spmd

    res = run_bass_kernel_spmd(nc, in_maps, list(range(N_CORES)))
    LAST_RESULTS = res

    y = np.zeros((N_AGENTS, 2), np.float32)
    for c in range(N_CORES):
        o = res.results[c]["outd"]
        sm = slotmaps[c]
        for h in (0, 1):
            m = sm[h] >= 0
            y[sm[h][m]] = o[2 * h : 2 * h + 2, m].T
    return y
